# revision 34
# baseline (speedup 1.0000x reference)
"""MoE (token-choice top-2 router + grouped SwiGLU experts + shared expert)
on 8 Trainium2 NeuronCores.

Sharding: expert-parallel with 2-segment load balancing. The host routes
(gate matmul, top-2, stable sort, gather/scale, final scatter-add); the
device does all FLOPs in bf16 with fp32 PSUM accumulation.

Load balancing (SPMD-uniform): every core runs the same program with TWO
routed segments of fixed sizes (S1, S2), S1+S2 ~ mean tokens/core. Each
segment's expert weights are per-core inputs, so the largest expert is
split across two cores' S1 slots, the smallest across two S2 slots, and
the remaining experts take one (S1, S2) pair each. This cuts the routed
capacity from max_e(count) (1083 for the seeded input) to ~1036, saving
~10us of matmul per core. Falls back to one capacity-sized segment if
the 2-segment packing is infeasible.

Trace-driven design notes (v2-v5, from a 397us composable-kernel
baseline; ~372us in the device's fast-clock state, x1.2 in its slow
DVFS state):
- ALL operands are SBUF-resident before use; weights arrive via
  host-swizzled DRAM layouts whose DMAs move 4-12KB contiguous
  per-partition lines (256B-packet streaming starved the PE).
- Weight chunk layout [P, mi, ks, 128]: the k-sweep for a fixed m-tile
  reads contiguous 256B blocks (strided LDWEIGHTS measured slower PE
  streaming - though the device's bimodal DVFS state confounds single
  measurements).
- Weight tensors stream through a 7-slot rotating tile pool (tag-level
  rotation = automatic prefetch pipelining via tile deps).
- Custom per-phase loops; r1 is n-outer (snaked chunk order across
  n-passes to avoid slot-recycle waits) so the first x n-chunk + first
  w1 m-tile (per-m-tile split DMAs) unblock the PE ~14us in; 8-bank
  PSUM rotation; outputs staged bf16 in SBUF, written as swizzled
  big-line DMAs (host unswizzles); the final out_s flush is per-m-tile
  with the psum copy split across scalar+vector.
- Remaining time is at architectural floors: ~7.4us engine-boot
  preamble, ~7us HBM-arrival-bound head, ~3% PE LDWEIGHTS-handoff
  overhead, ~3us end drain.

Self-contained: only needs numpy/ml_dtypes/concourse (the Bass stack).
"""

import math
import os

import numpy as np
import ml_dtypes

BF16 = ml_dtypes.bfloat16
NCORES = 8
TOP_K = 2
ROUTE_SCALE = 1.0
P = 128
WCH = 4  # m-chunks per weight tensor

# filled by the last kernel() call (exec_time_ns etc. when tracing)
LAST = {}

_PROGRAM_CACHE = {}


def _install_profhook():
    """Best-effort shim for antenv.axon_hooks so trace=True can capture NTFF
    profiles in this container. Harmless no-op if anything is missing."""
    try:
        import sys
        import types

        if "antenv.axon_hooks" in sys.modules:
            return
        import trn_agent_boot.trn_boot as tb

        hook = tb._ntff_profile_via_ctypes("/opt/axon/libaxon_pjrt.so")
        m = types.ModuleType("antenv.axon_hooks")
        m._hook = hook
        m.set_axon_ntff_profile_hook = lambda h: setattr(m, "_hook", h)
        m.get_axon_ntff_profile_hook = lambda: m._hook
        import antenv

        sys.modules["antenv.axon_hooks"] = m
        antenv.axon_hooks = m

        import concourse.bass_utils as bu

        bu.upload_artifacts = lambda tmpdir: tmpdir
    except Exception:
        pass


def _free_div(n):
    """Largest f = n/k (k<=4) with f <= 512, preferring big f."""
    for k in (1, 2, 3, 4):
        if n % k == 0 and n // k <= 512:
            return n // k
    for f in (512, 384, 256, 128):
        if n % f == 0:
            return f
    raise ValueError(f"no free-dim divisor for {n}")


def _pick_ntok(nmax, cap):
    """Smallest n in [nmax, cap] whose free-dim divides nicely (PSUM <=512)."""
    for n in range(nmax, cap + 1):
        try:
            _free_div(n)
            return n
        except ValueError:
            continue
    return cap


def _mchunks(n_mtiles, n_chunks):
    """Split n_mtiles 128-col m-tiles into n_chunks contiguous chunks,
    smallest chunk FIRST (the first chunk gates the kernel head)."""
    base = n_mtiles // n_chunks
    rem = n_mtiles % n_chunks
    sizes = [base] * (n_chunks - rem) + [base + 1] * rem
    out = []
    s = 0
    for sz in sizes:
        out.append((s, sz))
        s += sz
    return out


def _plan_segments(counts, ntok_single):
    """2-segment balanced packing: segment sizes (S1, S2) uniform across
    cores; per-core piece list [(expert, start, len), ...] per segment.
    Returns (segs, assign) or None if infeasible / not profitable."""
    E = len(counts)
    if E != NCORES or E < 3:
        return None
    order = sorted(range(E), key=lambda e: -counts[e])
    emax, emin = order[0], order[-1]
    others = order[1:-1]
    nmax, nmin = int(counts[emax]), int(counts[emin])
    try:
        S1 = _pick_ntok(max(128, (nmax + 1) // 2), max(128, (nmax + 1) // 2) + 127)
        need2 = max((nmin + 1) // 2, max(int(counts[e]) for e in others) - S1)
        S2 = _pick_ntok(max(128, need2), max(128, need2) + 127)
    except Exception:
        return None
    if S1 + S2 >= ntok_single or nmax > 2 * S1 or nmin > 2 * S2:
        return None
    if any(int(counts[e]) > S1 + S2 for e in others):
        return None
    la, lb = (nmax + 1) // 2, (nmin + 1) // 2
    assign = [
        [(emax, 0, la), (emin, 0, lb)],
        [(emax, la, nmax - la), (emin, lb, nmin - lb)],
    ]
    for e in others:
        sp = min(S1, int(counts[e]))
        assign.append([(e, 0, sp), (e, sp, int(counts[e]) - sp)])
    return [S1, S2], assign


def _build_program(D, H, SEGS, TS):
    import concourse.bacc as bacc
    import concourse.tile as tile
    from concourse import mybir
    from contextlib import ExitStack

    bf = mybir.dt.bfloat16
    f32 = mybir.dt.float32

    KD = D // P  # k-subtiles for the D-contraction (up-proj)
    KH = H // P  # k-subtiles for the H-contraction (out-proj)
    MT_H = H // P
    MT_D = D // P
    FREE = [_free_div(S) for S in SEGS]
    NCH = [S // f for S, f in zip(SEGS, FREE)]
    FREE_S = _free_div(TS)
    NCH_S = TS // FREE_S
    XKC = 2  # k-chunks for the xr prefetch (head latency)
    assert KD % XKC == 0
    KDC = KD // XKC
    NS = len(SEGS)

    up_chunks = _mchunks(MT_H, WCH)
    dn_chunks = _mchunks(MT_D, WCH)
    msz_max = max(sz for _, sz in dn_chunks)

    nc = bacc.Bacc(target_bir_lowering=False)

    # --- DRAM tensors (all host-swizzled; per-partition-contiguous lines) ---
    def wdecl(name, nk, chunks):
        return [
            nc.dram_tensor(f"{name}{c}", [P, nk * sz * P], bf, kind="ExternalInput")
            for c, (_, sz) in enumerate(chunks)
        ]

    w1d = [wdecl(f"w1s{s}", KD, up_chunks) for s in range(NS)]
    w3d = [wdecl(f"w3s{s}", KD, up_chunks) for s in range(NS)]
    sw1d = wdecl("sw1", KD, up_chunks)
    sw3d = wdecl("sw3", KD, up_chunks)
    w2d = [wdecl(f"w2s{s}", KH, dn_chunks) for s in range(NS)]
    sw2d = wdecl("sw2", KH, dn_chunks)
    xrd = [
        [
            [
                nc.dram_tensor(
                    f"xr{s}_{n}_{kc}", [P, KDC * FREE[s]], bf, kind="ExternalInput"
                )
                for kc in range(XKC)
            ]
            for n in range(NCH[s])
        ]
        for s in range(NS)
    ]
    xsd = nc.dram_tensor("xs", [P, KD * TS], bf, kind="ExternalInput")
    outr = [
        nc.dram_tensor(
            f"outr{s}", [WCH, NCH[s], P, msz_max * FREE[s]], bf, kind="ExternalOutput"
        )
        for s in range(NS)
    ]
    outs = nc.dram_tensor(
        "outs", [WCH, NCH_S, P, msz_max * FREE_S], bf, kind="ExternalOutput"
    )

    with tile.TileContext(nc) as tc, ExitStack() as ctx:
        caches = ctx.enter_context(tc.tile_pool(name="caches", bufs=1))
        xr_t = [
            [
                [
                    caches.tile(
                        [P, KDC, FREE[s]],
                        bf,
                        tag=f"xr{s}_{n}_{kc}",
                        name=f"xr{s}_{n}_{kc}",
                    )
                    for kc in range(XKC)
                ]
                for n in range(NCH[s])
            ]
            for s in range(NS)
        ]
        xs_t = caches.tile([P, KD, TS], bf, tag="xs")
        h1c = [
            caches.tile([P, MT_H, SEGS[s]], bf, tag=f"h1c{s}", name=f"h1c{s}")
            for s in range(NS)
        ]
        h1s = caches.tile([P, MT_H, TS], bf, tag="h1s")

        wpool = ctx.enter_context(tc.tile_pool(name="wpool", bufs=7))
        psum = ctx.enter_context(tc.tile_pool(name="psum", bufs=8, space="PSUM"))
        stgp = ctx.enter_context(tc.tile_pool(name="stg", bufs=1))

        # ---- prefetch issues (engine FIFO order = pacing) ----
        # scalar queue: xr segment/n-chunks in consumption order, then xs.
        # The very first n-chunk loads in ks-halves: subtile deps then let
        # the first k-sweep start after a 0.37MB transfer instead of 0.74MB
        # (the DMA subsystem ramps slowly in the first ~6us).
        for s in range(NS):
            for n in range(NCH[s]):
                for kc in range(XKC):
                    src = xrd[s][n][kc][:].rearrange("p (k j) -> p k j", k=KDC)
                    if s == 0 and n == 0:
                        hk = KDC // 2
                        nc.scalar.dma_start(
                            out=xr_t[s][n][kc][:, :hk], in_=src[:, :hk]
                        )
                        nc.scalar.dma_start(
                            out=xr_t[s][n][kc][:, hk:], in_=src[:, hk:]
                        )
                    else:
                        nc.scalar.dma_start(out=xr_t[s][n][kc][:], in_=src)
        nc.scalar.dma_start(
            out=xs_t[:], in_=xsd[:].rearrange("p (k j) -> p k j", k=KD)
        )

        # sync queue: weight chunks in consumption order; 7-slot tag
        # rotation = automatic prefetch pipeline via tile deps.
        def wload(dram_chunks, nk, chunks, label, split0=False):
            tiles = []
            for c, (_, sz) in enumerate(chunks):
                t = wpool.tile([P, sz, nk, P], bf, tag="w", name=f"{label}{c}")
                src = dram_chunks[c][:].rearrange("p (m k j) -> p m k j", m=sz, k=nk)
                if split0 and c == 0:
                    for mi in range(sz):
                        nc.sync.dma_start(
                            out=t[:, mi : mi + 1], in_=src[:, mi : mi + 1]
                        )
                else:
                    nc.sync.dma_start(out=t[:], in_=src)
                tiles.append(t)
            return tiles

        w1t = [
            wload(w1d[s], KD, up_chunks, f"w1s{s}t", split0=(s == 0))
            for s in range(NS)
        ]
        w3t = [wload(w3d[s], KD, up_chunks, f"w3s{s}t") for s in range(NS)]
        sw1t = wload(sw1d, KD, up_chunks, "sw1t")
        sw3t = wload(sw3d, KD, up_chunks, "sw3t")
        w2t = [wload(w2d[s], KH, dn_chunks, f"w2s{s}t") for s in range(NS)]
        sw2t = wload(sw2d, KH, dn_chunks, "sw2t")

        Silu = mybir.ActivationFunctionType.Silu

        def xr_rhs(s, n, ks):
            return xr_t[s][n][ks // KDC][:, ks % KDC, :]

        # ---- phase r1: h1 = silu(w1T.T @ xr), n-outer (stream-friendly);
        # snake the chunk order across n-passes so each pass resumes on the
        # chunk whose tile deps were satisfied most recently ----
        for s in range(NS):
            for n in range(NCH[s]):
                order = list(enumerate(up_chunks))
                if n % 2 == 1:
                    order = order[::-1]
                for c, (m0, msz) in order:
                    for mi in range(msz):
                        ps = psum.tile(
                            [P, 512], f32, tag="ps", name=f"ps_r1_{s}_{n}_{c}_{mi}"
                        )
                        for ks in range(KD):
                            nc.tensor.matmul(
                                ps[:, : FREE[s]],
                                w1t[s][c][:, mi, ks, :],
                                xr_rhs(s, n, ks),
                                start=(ks == 0),
                                stop=(ks == KD - 1),
                            )
                        nc.scalar.activation(
                            h1c[s][:, m0 + mi, n * FREE[s] : (n + 1) * FREE[s]],
                            ps[:, : FREE[s]],
                            Silu,
                        )

        # ---- phase r3: h1 *= (w3T.T @ xr), n-inner (weight reuse) ----
        for s in range(NS):
            for c, (m0, msz) in enumerate(up_chunks):
                for mi in range(msz):
                    pss = [
                        psum.tile([P, 512], f32, tag="ps", name=f"ps_r3_{s}_{c}_{mi}_{n}")
                        for n in range(NCH[s])
                    ]
                    for ks in range(KD):
                        for n in range(NCH[s]):
                            nc.tensor.matmul(
                                pss[n][:, : FREE[s]],
                                w3t[s][c][:, mi, ks, :],
                                xr_rhs(s, n, ks),
                                start=(ks == 0),
                                stop=(ks == KD - 1),
                            )
                    for n in range(NCH[s]):
                        sl = h1c[s][:, m0 + mi, n * FREE[s] : (n + 1) * FREE[s]]
                        nc.vector.tensor_mul(out=sl, in0=pss[n][:, : FREE[s]], in1=sl)

        # ---- phase s1/s3: shared-expert swiglu on xs ----
        for wt, is_mul in ((sw1t, False), (sw3t, True)):
            for c, (m0, msz) in enumerate(up_chunks):
                for mi in range(msz):
                    pss = [
                        psum.tile([P, 512], f32, tag="ps", name=f"ps_s_{c}_{mi}_{n}")
                        for n in range(NCH_S)
                    ]
                    for ks in range(KD):
                        for n in range(NCH_S):
                            nc.tensor.matmul(
                                pss[n][:, :FREE_S],
                                wt[c][:, mi, ks, :],
                                xs_t[:, ks, n * FREE_S : (n + 1) * FREE_S],
                                start=(ks == 0),
                                stop=(ks == KD - 1),
                            )
                    for n in range(NCH_S):
                        sl = h1s[:, m0 + mi, n * FREE_S : (n + 1) * FREE_S]
                        if is_mul:
                            nc.vector.tensor_mul(
                                out=sl, in0=pss[n][:, :FREE_S], in1=sl
                            )
                        else:
                            nc.scalar.activation(sl, pss[n][:, :FREE_S], Silu)

        # ---- phase out_r: outrT = w2T.T @ h1 (n-inner; vector copies,
        # scalar DMAs) ----
        for s in range(NS):
            for c, (m0, msz) in enumerate(dn_chunks):
                stgs = [
                    stgp.tile(
                        [P, msz, FREE[s]],
                        bf,
                        tag=f"stgr{s}",
                        bufs=4,
                        name=f"stgr{s}_{c}_{n}",
                    )
                    for n in range(NCH[s])
                ]
                for mi in range(msz):
                    pss = [
                        psum.tile(
                            [P, 512], f32, tag="ps", name=f"ps_or_{s}_{c}_{mi}_{n}"
                        )
                        for n in range(NCH[s])
                    ]
                    for ks in range(KH):
                        for n in range(NCH[s]):
                            nc.tensor.matmul(
                                pss[n][:, : FREE[s]],
                                w2t[s][c][:, mi, ks, :],
                                h1c[s][:, ks, n * FREE[s] : (n + 1) * FREE[s]],
                                start=(ks == 0),
                                stop=(ks == KH - 1),
                            )
                    for n in range(NCH[s]):
                        nc.vector.tensor_copy(
                            out=stgs[n][:, mi, :], in_=pss[n][:, : FREE[s]]
                        )
                for n in range(NCH[s]):
                    nc.scalar.dma_start(
                        out=outr[s][c, n, :, : msz * FREE[s]].rearrange(
                            "p (m j) -> p m j", m=msz
                        ),
                        in_=stgs[n][:],
                    )

        # ---- phase out_s: outsT = sw2T.T @ h1s (split copies, sync DMAs) ----
        for c, (m0, msz) in enumerate(dn_chunks):
            stgs = [
                stgp.tile([P, msz, FREE_S], bf, tag="stgs", bufs=2, name=f"stgs{c}_{n}")
                for n in range(NCH_S)
            ]
            for mi in range(msz):
                pss = [
                    psum.tile([P, 512], f32, tag="ps", name=f"ps_os_{c}_{mi}_{n}")
                    for n in range(NCH_S)
                ]
                for ks in range(KH):
                    for n in range(NCH_S):
                        nc.tensor.matmul(
                            pss[n][:, :FREE_S],
                            sw2t[c][:, mi, ks, :],
                            h1s[:, ks, n * FREE_S : (n + 1) * FREE_S],
                            start=(ks == 0),
                            stop=(ks == KH - 1),
                        )
                for n in range(NCH_S):
                    # halve the copy latency on the end-of-kernel critical
                    # chain: scalar and vector each copy half the tile
                    hf = FREE_S // 2
                    nc.scalar.activation(
                        stgs[n][:, mi, :hf],
                        pss[n][:, :hf],
                        mybir.ActivationFunctionType.Copy,
                    )
                    nc.vector.tensor_copy(
                        out=stgs[n][:, mi, hf:FREE_S], in_=pss[n][:, hf:FREE_S]
                    )
                # per-mi output DMA: the final flush after the last matmul
                # is one m-tile, not a whole chunk (shrinks the kernel tail)
                for n in range(NCH_S):
                    nc.sync.dma_start(
                        out=outs[c, n, :, : msz * FREE_S].rearrange(
                            "p (m j) -> p m j", m=msz
                        )[:, mi : mi + 1],
                        in_=stgs[n][:, mi : mi + 1],
                    )

    nc.compile()
    return nc


def _route(x, gate_w, expert_bias):
    """Host control plane mirroring the reference routing exactly."""
    BS, SLEN, D = x.shape
    T = BS * SLEN
    xt = np.ascontiguousarray(x.reshape(T, D), dtype=np.float32)
    logits = xt @ gate_w.astype(np.float32).T  # [T, E]
    scores = 1.0 / (1.0 + np.exp(-logits))
    biased = scores + np.asarray(expert_bias, np.float32)[None, :]
    sel = np.argsort(-biased, axis=1, kind="stable")[:, :TOP_K]  # [T, K]
    top_scores = np.take_along_axis(scores, sel, axis=1) * ROUTE_SCALE
    sel_flat = sel.reshape(-1)
    order = np.argsort(sel_flat, kind="stable")  # [T*K]
    counts = np.bincount(sel_flat, minlength=NCORES)
    tok_idx = order // TOP_K
    scores_sorted = top_scores.reshape(-1)[order].astype(np.float32)
    return xt, counts, tok_idx, scores_sorted


def _swz_w(wT, nk, chunks):
    """wT [K, M] f32 -> list of [P, sz*nk*P] bf16 swizzled chunks with
    per-partition layout (mi, ks, j): the device k-sweep for a fixed m-tile
    reads contiguous 256B blocks."""
    K, M = wT.shape
    w3d = wT.reshape(nk, P, M)  # (ks, p, m)
    out = []
    for m0, sz in chunks:
        blk = w3d[:, :, m0 * P : (m0 + sz) * P].reshape(nk, P, sz, P)
        # (ks, p, mi, j) -> (p, mi, ks, j)
        out.append(
            np.ascontiguousarray(blk.transpose(1, 2, 0, 3))
            .reshape(P, sz * nk * P)
            .astype(BF16)
        )
    return out


def _swz_x(xrT, KDC, XKC, free, nch):
    """xrT [D, S] f32 -> dict of [P, KDC*free] bf16 chunks keyed (n, kc)."""
    D, S = xrT.shape
    xr4 = xrT.reshape(XKC, KDC, P, S)  # (kc, ks, p, tok)
    out = {}
    for n in range(nch):
        for kc in range(XKC):
            blk = xr4[kc, :, :, n * free : (n + 1) * free]
            out[(n, kc)] = (
                np.ascontiguousarray(blk.transpose(1, 0, 2))
                .reshape(P, KDC * free)
                .astype(BF16)
            )
    return out


def kernel(x, gate_w, w1, w2, w3, sw1, sw2, sw3, expert_bias):
    from concourse.bass_utils import run_bass_kernel_spmd

    x = np.asarray(x, np.float32)
    gate_w = np.asarray(gate_w, np.float32)
    w1 = np.asarray(w1, np.float32)
    w2 = np.asarray(w2, np.float32)
    w3 = np.asarray(w3, np.float32)
    sw1 = np.asarray(sw1, np.float32)
    sw2 = np.asarray(sw2, np.float32)
    sw3 = np.asarray(sw3, np.float32)
    expert_bias = np.asarray(expert_bias, np.float32)
    BS, SLEN, D = x.shape
    T = BS * SLEN
    H = w1.shape[1]
    TS = T // NCORES
    KD = D // P
    KH = H // P
    MT_D = D // P
    XKC = 2
    KDC = KD // XKC
    up_chunks = _mchunks(H // P, WCH)
    dn_chunks = _mchunks(MT_D, WCH)
    msz_max = max(sz for _, sz in dn_chunks)

    xt, counts, tok_idx, scores_sorted = _route(x, gate_w, expert_bias)
    off = np.concatenate([[0], np.cumsum(counts)]).astype(np.int64)
    CAP = max(128, int(math.ceil(counts.max() / 128) * 128))
    NTOK = _pick_ntok(max(128, int(counts.max())), CAP)

    plan = _plan_segments(counts, NTOK)
    if plan is not None:
        SEGS, assign = plan
    else:
        SEGS = [NTOK]
        assign = [[(e, 0, int(counts[e]))] for e in range(NCORES)]
    NS = len(SEGS)
    FREE = [_free_div(S) for S in SEGS]
    NCH = [S // f for S, f in zip(SEGS, FREE)]
    FREE_S = _free_div(TS)
    NCH_S = TS // FREE_S

    key = (D, H, tuple(SEGS), TS)
    if key not in _PROGRAM_CACHE:
        _PROGRAM_CACHE[key] = _build_program(D, H, SEGS, TS)
    nc = _PROGRAM_CACHE[key]

    # ---- stage per-core inputs (swizzled; expert weight swizzles memoized
    # since a split expert appears on two cores) ----
    shared_chunks = {}
    for name, wmat, nk, chunks in (
        ("sw1", sw1.T, KD, up_chunks),
        ("sw3", sw3.T, KD, up_chunks),
        ("sw2", sw2.T, KH, dn_chunks),
    ):
        for c, arr in enumerate(_swz_w(np.ascontiguousarray(wmat), nk, chunks)):
            shared_chunks[f"{name}{c}"] = arr

    wmemo = {}

    def expert_chunks(kind, e):
        k = (kind, e)
        if k not in wmemo:
            mat, nk, chunks = {
                "w1": (w1[e].T, KD, up_chunks),
                "w3": (w3[e].T, KD, up_chunks),
                "w2": (w2[e].T, KH, dn_chunks),
            }[kind]
            wmemo[k] = _swz_w(np.ascontiguousarray(mat), nk, chunks)
        return wmemo[k]

    in_maps = []
    for core in range(NCORES):
        im = dict(shared_chunks)
        for s, (e, start, ln) in enumerate(assign[core]):
            seg = SEGS[s]
            idx = tok_idx[off[e] + start : off[e] + start + ln]
            xrT = np.zeros((D, seg), np.float32)
            xrT[:, :ln] = (
                xt[idx] * scores_sorted[off[e] + start : off[e] + start + ln, None]
            ).T
            for (n, kc), arr in _swz_x(xrT, KDC, XKC, FREE[s], NCH[s]).items():
                im[f"xr{s}_{n}_{kc}"] = arr
            for kind in ("w1", "w3", "w2"):
                for c, arr in enumerate(expert_chunks(kind, e)):
                    im[f"{kind}s{s}{c}"] = arr
        xsT = np.ascontiguousarray(xt[core * TS : (core + 1) * TS].T)
        im["xs"] = (
            xsT.reshape(KD, P, TS).transpose(1, 0, 2).reshape(P, KD * TS).astype(BF16)
        )
        in_maps.append(im)

    trace = os.environ.get("KERNEL_TRACE", "") not in ("", "0")
    if trace:
        _install_profhook()
    res = run_bass_kernel_spmd(nc, in_maps, list(range(NCORES)), trace=trace)
    LAST["exec_time_ns"] = res.exec_time_ns
    LAST["results"] = res

    # ---- combine: unswizzle outputs, shared slices + routed scatter-add ----
    def unswz(arr, nch, free):
        # arr [WCH, nch, P, msz_max*free] -> [D, nch*free] f32
        full = np.empty((MT_D * P, nch * free), np.float32)
        for c, (m0, sz) in enumerate(dn_chunks):
            blk = np.asarray(arr[c, :, :, : sz * free], dtype=np.float32).reshape(
                nch, P, sz, free
            )
            full[m0 * P : (m0 + sz) * P] = blk.transpose(2, 1, 0, 3).reshape(
                sz * P, nch * free
            )
        return full

    out = np.empty((T, D), np.float32)
    for core in range(NCORES):
        osw = np.asarray(res.results[core]["outs"])
        out[core * TS : (core + 1) * TS] = unswz(osw, NCH_S, FREE_S).T
    for core in range(NCORES):
        for s, (e, start, ln) in enumerate(assign[core]):
            if ln <= 0:
                continue
            orw = np.asarray(res.results[core][f"outr{s}"])
            full = unswz(orw, NCH[s], FREE[s])  # [D, SEGS[s]]
            idx = tok_idx[off[e] + start : off[e] + start + ln]
            out[idx] += full[:, :ln].T
    return out.reshape(BS, SLEN, D)


# revision 35
# speedup vs baseline: 1.0055x; 1.0055x over previous
"""MoE (token-choice top-2 router + grouped SwiGLU experts + shared expert)
on 8 Trainium2 NeuronCores.

Sharding: expert-parallel with 2-segment load balancing. The host routes
(gate matmul, top-2, stable sort, gather/scale, final scatter-add); the
device does all FLOPs in bf16 with fp32 PSUM accumulation.

Load balancing (SPMD-uniform): every core runs the same program with TWO
routed segments of fixed sizes (S1, S2), S1+S2 ~ mean tokens/core. Each
segment's expert weights are per-core inputs, so the largest expert is
split across two cores' S1 slots, the smallest across two S2 slots, and
the remaining experts take one (S1, S2) pair each. This cuts the routed
capacity from max_e(count) (1083 for the seeded input) to ~1036, saving
~10us of matmul per core. Falls back to one capacity-sized segment if
the 2-segment packing is infeasible.

Trace-driven design notes (v2-v5, from a 397us composable-kernel
baseline; ~372us in the device's fast-clock state, x1.2 in its slow
DVFS state):
- ALL operands are SBUF-resident before use; weights arrive via
  host-swizzled DRAM layouts whose DMAs move 4-12KB contiguous
  per-partition lines (256B-packet streaming starved the PE).
- Weight chunk layout [P, mi, ks, 128]: the k-sweep for a fixed m-tile
  reads contiguous 256B blocks (strided LDWEIGHTS measured slower PE
  streaming - though the device's bimodal DVFS state confounds single
  measurements).
- Weight tensors stream through a 7-slot rotating tile pool (tag-level
  rotation = automatic prefetch pipelining via tile deps).
- Custom per-phase loops; r1 is n-outer (snaked chunk order across
  n-passes to avoid slot-recycle waits) so the first x n-chunk + first
  w1 m-tile (per-m-tile split DMAs) unblock the PE ~14us in; 8-bank
  PSUM rotation; outputs staged bf16 in SBUF, written as swizzled
  big-line DMAs (host unswizzles); the final out_s flush is per-m-tile
  with the psum copy split across scalar+vector.
- Remaining time is at architectural floors: ~7.4us engine-boot
  preamble, ~7us HBM-arrival-bound head, ~3% PE LDWEIGHTS-handoff
  overhead, ~3us end drain.

Self-contained: only needs numpy/ml_dtypes/concourse (the Bass stack).
"""

import math
import os

import numpy as np
import ml_dtypes

BF16 = ml_dtypes.bfloat16
NCORES = 8
TOP_K = 2
ROUTE_SCALE = 1.0
P = 128
WCH = 4  # m-chunks per weight tensor

# filled by the last kernel() call (exec_time_ns etc. when tracing)
LAST = {}

_PROGRAM_CACHE = {}


def _install_profhook():
    """Best-effort shim for antenv.axon_hooks so trace=True can capture NTFF
    profiles in this container. Harmless no-op if anything is missing."""
    try:
        import sys
        import types

        if "antenv.axon_hooks" in sys.modules:
            return
        import trn_agent_boot.trn_boot as tb

        hook = tb._ntff_profile_via_ctypes("/opt/axon/libaxon_pjrt.so")
        m = types.ModuleType("antenv.axon_hooks")
        m._hook = hook
        m.set_axon_ntff_profile_hook = lambda h: setattr(m, "_hook", h)
        m.get_axon_ntff_profile_hook = lambda: m._hook
        import antenv

        sys.modules["antenv.axon_hooks"] = m
        antenv.axon_hooks = m

        import concourse.bass_utils as bu

        bu.upload_artifacts = lambda tmpdir: tmpdir
    except Exception:
        pass


def _free_div(n):
    """Largest f = n/k (k<=4) with f <= 512, preferring big f."""
    for k in (1, 2, 3, 4):
        if n % k == 0 and n // k <= 512:
            return n // k
    for f in (512, 384, 256, 128):
        if n % f == 0:
            return f
    raise ValueError(f"no free-dim divisor for {n}")


def _pick_ntok(nmax, cap):
    """Smallest n in [nmax, cap] whose free-dim divides nicely (PSUM <=512)."""
    for n in range(nmax, cap + 1):
        try:
            _free_div(n)
            return n
        except ValueError:
            continue
    return cap


def _mchunks(n_mtiles, n_chunks):
    """Split n_mtiles 128-col m-tiles into n_chunks contiguous chunks,
    smallest chunk FIRST (the first chunk gates the kernel head)."""
    base = n_mtiles // n_chunks
    rem = n_mtiles % n_chunks
    sizes = [base] * (n_chunks - rem) + [base + 1] * rem
    out = []
    s = 0
    for sz in sizes:
        out.append((s, sz))
        s += sz
    return out


def _plan_segments(counts, ntok_single):
    """2-segment balanced packing: segment sizes (S1, S2) uniform across
    cores; per-core piece list [(expert, start, len), ...] per segment.
    Returns (segs, assign) or None if infeasible / not profitable."""
    E = len(counts)
    if E != NCORES or E < 3:
        return None
    order = sorted(range(E), key=lambda e: -counts[e])
    emax, emin = order[0], order[-1]
    others = order[1:-1]
    nmax, nmin = int(counts[emax]), int(counts[emin])
    try:
        S1 = _pick_ntok(max(128, (nmax + 1) // 2), max(128, (nmax + 1) // 2) + 127)
        need2 = max((nmin + 1) // 2, max(int(counts[e]) for e in others) - S1)
        S2 = _pick_ntok(max(128, need2), max(128, need2) + 127)
    except Exception:
        return None
    if S1 + S2 >= ntok_single or nmax > 2 * S1 or nmin > 2 * S2:
        return None
    if any(int(counts[e]) > S1 + S2 for e in others):
        return None
    la, lb = (nmax + 1) // 2, (nmin + 1) // 2
    assign = [
        [(emax, 0, la), (emin, 0, lb)],
        [(emax, la, nmax - la), (emin, lb, nmin - lb)],
    ]
    for e in others:
        sp = min(S1, int(counts[e]))
        assign.append([(e, 0, sp), (e, sp, int(counts[e]) - sp)])
    return [S1, S2], assign


def _build_program(D, H, SEGS, TS):
    import concourse.bacc as bacc
    import concourse.tile as tile
    from concourse import mybir
    from contextlib import ExitStack

    bf = mybir.dt.bfloat16
    f32 = mybir.dt.float32

    KD = D // P  # k-subtiles for the D-contraction (up-proj)
    KH = H // P  # k-subtiles for the H-contraction (out-proj)
    MT_H = H // P
    MT_D = D // P
    FREE = [_free_div(S) for S in SEGS]
    NCH = [S // f for S, f in zip(SEGS, FREE)]
    FREE_S = _free_div(TS)
    NCH_S = TS // FREE_S
    XKC = 2  # k-chunks for the xr prefetch (head latency)
    assert KD % XKC == 0
    KDC = KD // XKC
    NS = len(SEGS)

    up_chunks = _mchunks(MT_H, WCH)
    dn_chunks = _mchunks(MT_D, WCH)
    msz_max = max(sz for _, sz in dn_chunks)

    nc = bacc.Bacc(target_bir_lowering=False)

    # --- DRAM tensors (all host-swizzled; per-partition-contiguous lines) ---
    def wdecl(name, nk, chunks):
        return [
            nc.dram_tensor(f"{name}{c}", [P, nk * sz * P], bf, kind="ExternalInput")
            for c, (_, sz) in enumerate(chunks)
        ]

    w1d = [wdecl(f"w1s{s}", KD, up_chunks) for s in range(NS)]
    w3d = [wdecl(f"w3s{s}", KD, up_chunks) for s in range(NS)]
    sw1d = wdecl("sw1", KD, up_chunks)
    sw3d = wdecl("sw3", KD, up_chunks)
    w2d = [wdecl(f"w2s{s}", KH, dn_chunks) for s in range(NS)]
    sw2d = wdecl("sw2", KH, dn_chunks)
    xrd = [
        [
            [
                nc.dram_tensor(
                    f"xr{s}_{n}_{kc}", [P, KDC * FREE[s]], bf, kind="ExternalInput"
                )
                for kc in range(XKC)
            ]
            for n in range(NCH[s])
        ]
        for s in range(NS)
    ]
    xsd = nc.dram_tensor("xs", [P, KD * TS], bf, kind="ExternalInput")
    outr = [
        nc.dram_tensor(
            f"outr{s}", [WCH, NCH[s], P, msz_max * FREE[s]], bf, kind="ExternalOutput"
        )
        for s in range(NS)
    ]
    outs = nc.dram_tensor(
        "outs", [WCH, NCH_S, P, msz_max * FREE_S], bf, kind="ExternalOutput"
    )

    with tile.TileContext(nc) as tc, ExitStack() as ctx:
        caches = ctx.enter_context(tc.tile_pool(name="caches", bufs=1))
        xr_t = [
            [
                [
                    caches.tile(
                        [P, KDC, FREE[s]],
                        bf,
                        tag=f"xr{s}_{n}_{kc}",
                        name=f"xr{s}_{n}_{kc}",
                    )
                    for kc in range(XKC)
                ]
                for n in range(NCH[s])
            ]
            for s in range(NS)
        ]
        xs_t = caches.tile([P, KD, TS], bf, tag="xs")
        h1c = [
            caches.tile([P, MT_H, SEGS[s]], bf, tag=f"h1c{s}", name=f"h1c{s}")
            for s in range(NS)
        ]
        h1s = caches.tile([P, MT_H, TS], bf, tag="h1s")

        wpool = ctx.enter_context(tc.tile_pool(name="wpool", bufs=7))
        psum = ctx.enter_context(tc.tile_pool(name="psum", bufs=8, space="PSUM"))
        stgp = ctx.enter_context(tc.tile_pool(name="stg", bufs=1))

        # ---- prefetch issues (engine FIFO order = pacing) ----
        # scalar queue: xr segment/n-chunks in consumption order, then xs
        # (finer first-piece splits measured net-negative: issue cost and
        # trickle stalls outweigh the earlier PE start)
        for s in range(NS):
            for n in range(NCH[s]):
                for kc in range(XKC):
                    nc.scalar.dma_start(
                        out=xr_t[s][n][kc][:],
                        in_=xrd[s][n][kc][:].rearrange("p (k j) -> p k j", k=KDC),
                    )
        nc.scalar.dma_start(
            out=xs_t[:], in_=xsd[:].rearrange("p (k j) -> p k j", k=KD)
        )

        # sync queue: weight chunks in consumption order; 7-slot tag
        # rotation = automatic prefetch pipeline via tile deps.
        def wload(dram_chunks, nk, chunks, label, split0=False):
            tiles = []
            for c, (_, sz) in enumerate(chunks):
                t = wpool.tile([P, sz, nk, P], bf, tag="w", name=f"{label}{c}")
                src = dram_chunks[c][:].rearrange("p (m k j) -> p m k j", m=sz, k=nk)
                if split0 and c == 0:
                    for mi in range(sz):
                        nc.sync.dma_start(
                            out=t[:, mi : mi + 1], in_=src[:, mi : mi + 1]
                        )
                else:
                    nc.sync.dma_start(out=t[:], in_=src)
                tiles.append(t)
            return tiles

        w1t = [
            wload(w1d[s], KD, up_chunks, f"w1s{s}t", split0=(s == 0))
            for s in range(NS)
        ]
        w3t = [wload(w3d[s], KD, up_chunks, f"w3s{s}t") for s in range(NS)]
        sw1t = wload(sw1d, KD, up_chunks, "sw1t")
        sw3t = wload(sw3d, KD, up_chunks, "sw3t")
        w2t = [wload(w2d[s], KH, dn_chunks, f"w2s{s}t") for s in range(NS)]
        sw2t = wload(sw2d, KH, dn_chunks, "sw2t")

        Silu = mybir.ActivationFunctionType.Silu

        def xr_rhs(s, n, ks):
            return xr_t[s][n][ks // KDC][:, ks % KDC, :]

        # ---- phase r1: h1 = silu(w1T.T @ xr), n-outer (stream-friendly);
        # snake the chunk order across n-passes so each pass resumes on the
        # chunk whose tile deps were satisfied most recently ----
        for s in range(NS):
            for n in range(NCH[s]):
                order = list(enumerate(up_chunks))
                if n % 2 == 1:
                    order = order[::-1]
                for c, (m0, msz) in order:
                    for mi in range(msz):
                        ps = psum.tile(
                            [P, 512], f32, tag="ps", name=f"ps_r1_{s}_{n}_{c}_{mi}"
                        )
                        for ks in range(KD):
                            nc.tensor.matmul(
                                ps[:, : FREE[s]],
                                w1t[s][c][:, mi, ks, :],
                                xr_rhs(s, n, ks),
                                start=(ks == 0),
                                stop=(ks == KD - 1),
                            )
                        nc.scalar.activation(
                            h1c[s][:, m0 + mi, n * FREE[s] : (n + 1) * FREE[s]],
                            ps[:, : FREE[s]],
                            Silu,
                        )

        # ---- phase r3: h1 *= (w3T.T @ xr), n-inner (weight reuse) ----
        for s in range(NS):
            for c, (m0, msz) in enumerate(up_chunks):
                for mi in range(msz):
                    pss = [
                        psum.tile([P, 512], f32, tag="ps", name=f"ps_r3_{s}_{c}_{mi}_{n}")
                        for n in range(NCH[s])
                    ]
                    for ks in range(KD):
                        for n in range(NCH[s]):
                            nc.tensor.matmul(
                                pss[n][:, : FREE[s]],
                                w3t[s][c][:, mi, ks, :],
                                xr_rhs(s, n, ks),
                                start=(ks == 0),
                                stop=(ks == KD - 1),
                            )
                    for n in range(NCH[s]):
                        sl = h1c[s][:, m0 + mi, n * FREE[s] : (n + 1) * FREE[s]]
                        nc.vector.tensor_mul(out=sl, in0=pss[n][:, : FREE[s]], in1=sl)

        # ---- phase s1/s3: shared-expert swiglu on xs ----
        for wt, is_mul in ((sw1t, False), (sw3t, True)):
            for c, (m0, msz) in enumerate(up_chunks):
                for mi in range(msz):
                    pss = [
                        psum.tile([P, 512], f32, tag="ps", name=f"ps_s_{c}_{mi}_{n}")
                        for n in range(NCH_S)
                    ]
                    for ks in range(KD):
                        for n in range(NCH_S):
                            nc.tensor.matmul(
                                pss[n][:, :FREE_S],
                                wt[c][:, mi, ks, :],
                                xs_t[:, ks, n * FREE_S : (n + 1) * FREE_S],
                                start=(ks == 0),
                                stop=(ks == KD - 1),
                            )
                    for n in range(NCH_S):
                        sl = h1s[:, m0 + mi, n * FREE_S : (n + 1) * FREE_S]
                        if is_mul:
                            nc.vector.tensor_mul(
                                out=sl, in0=pss[n][:, :FREE_S], in1=sl
                            )
                        else:
                            nc.scalar.activation(sl, pss[n][:, :FREE_S], Silu)

        # ---- phase out_r: outrT = w2T.T @ h1 (n-inner; vector copies,
        # scalar DMAs) ----
        for s in range(NS):
            for c, (m0, msz) in enumerate(dn_chunks):
                stgs = [
                    stgp.tile(
                        [P, msz, FREE[s]],
                        bf,
                        tag=f"stgr{s}",
                        bufs=4,
                        name=f"stgr{s}_{c}_{n}",
                    )
                    for n in range(NCH[s])
                ]
                for mi in range(msz):
                    pss = [
                        psum.tile(
                            [P, 512], f32, tag="ps", name=f"ps_or_{s}_{c}_{mi}_{n}"
                        )
                        for n in range(NCH[s])
                    ]
                    for ks in range(KH):
                        for n in range(NCH[s]):
                            nc.tensor.matmul(
                                pss[n][:, : FREE[s]],
                                w2t[s][c][:, mi, ks, :],
                                h1c[s][:, ks, n * FREE[s] : (n + 1) * FREE[s]],
                                start=(ks == 0),
                                stop=(ks == KH - 1),
                            )
                    for n in range(NCH[s]):
                        nc.vector.tensor_copy(
                            out=stgs[n][:, mi, :], in_=pss[n][:, : FREE[s]]
                        )
                for n in range(NCH[s]):
                    nc.scalar.dma_start(
                        out=outr[s][c, n, :, : msz * FREE[s]].rearrange(
                            "p (m j) -> p m j", m=msz
                        ),
                        in_=stgs[n][:],
                    )

        # ---- phase out_s: outsT = sw2T.T @ h1s (split copies, sync DMAs) ----
        for c, (m0, msz) in enumerate(dn_chunks):
            stgs = [
                stgp.tile([P, msz, FREE_S], bf, tag="stgs", bufs=2, name=f"stgs{c}_{n}")
                for n in range(NCH_S)
            ]
            for mi in range(msz):
                pss = [
                    psum.tile([P, 512], f32, tag="ps", name=f"ps_os_{c}_{mi}_{n}")
                    for n in range(NCH_S)
                ]
                for ks in range(KH):
                    for n in range(NCH_S):
                        nc.tensor.matmul(
                            pss[n][:, :FREE_S],
                            sw2t[c][:, mi, ks, :],
                            h1s[:, ks, n * FREE_S : (n + 1) * FREE_S],
                            start=(ks == 0),
                            stop=(ks == KH - 1),
                        )
                for n in range(NCH_S):
                    # halve the copy latency on the end-of-kernel critical
                    # chain: scalar and vector each copy half the tile
                    hf = FREE_S // 2
                    nc.scalar.activation(
                        stgs[n][:, mi, :hf],
                        pss[n][:, :hf],
                        mybir.ActivationFunctionType.Copy,
                    )
                    nc.vector.tensor_copy(
                        out=stgs[n][:, mi, hf:FREE_S], in_=pss[n][:, hf:FREE_S]
                    )
                # per-mi output DMA: the final flush after the last matmul
                # is one m-tile, not a whole chunk (shrinks the kernel tail)
                for n in range(NCH_S):
                    nc.sync.dma_start(
                        out=outs[c, n, :, : msz * FREE_S].rearrange(
                            "p (m j) -> p m j", m=msz
                        )[:, mi : mi + 1],
                        in_=stgs[n][:, mi : mi + 1],
                    )

    nc.compile()
    return nc


def _route(x, gate_w, expert_bias):
    """Host control plane mirroring the reference routing exactly."""
    BS, SLEN, D = x.shape
    T = BS * SLEN
    xt = np.ascontiguousarray(x.reshape(T, D), dtype=np.float32)
    logits = xt @ gate_w.astype(np.float32).T  # [T, E]
    scores = 1.0 / (1.0 + np.exp(-logits))
    biased = scores + np.asarray(expert_bias, np.float32)[None, :]
    sel = np.argsort(-biased, axis=1, kind="stable")[:, :TOP_K]  # [T, K]
    top_scores = np.take_along_axis(scores, sel, axis=1) * ROUTE_SCALE
    sel_flat = sel.reshape(-1)
    order = np.argsort(sel_flat, kind="stable")  # [T*K]
    counts = np.bincount(sel_flat, minlength=NCORES)
    tok_idx = order // TOP_K
    scores_sorted = top_scores.reshape(-1)[order].astype(np.float32)
    return xt, counts, tok_idx, scores_sorted


def _swz_w(wT, nk, chunks):
    """wT [K, M] f32 -> list of [P, sz*nk*P] bf16 swizzled chunks with
    per-partition layout (mi, ks, j): the device k-sweep for a fixed m-tile
    reads contiguous 256B blocks."""
    K, M = wT.shape
    w3d = wT.reshape(nk, P, M)  # (ks, p, m)
    out = []
    for m0, sz in chunks:
        blk = w3d[:, :, m0 * P : (m0 + sz) * P].reshape(nk, P, sz, P)
        # (ks, p, mi, j) -> (p, mi, ks, j)
        out.append(
            np.ascontiguousarray(blk.transpose(1, 2, 0, 3))
            .reshape(P, sz * nk * P)
            .astype(BF16)
        )
    return out


def _swz_x(xrT, KDC, XKC, free, nch):
    """xrT [D, S] f32 -> dict of [P, KDC*free] bf16 chunks keyed (n, kc)."""
    D, S = xrT.shape
    xr4 = xrT.reshape(XKC, KDC, P, S)  # (kc, ks, p, tok)
    out = {}
    for n in range(nch):
        for kc in range(XKC):
            blk = xr4[kc, :, :, n * free : (n + 1) * free]
            out[(n, kc)] = (
                np.ascontiguousarray(blk.transpose(1, 0, 2))
                .reshape(P, KDC * free)
                .astype(BF16)
            )
    return out


def kernel(x, gate_w, w1, w2, w3, sw1, sw2, sw3, expert_bias):
    from concourse.bass_utils import run_bass_kernel_spmd

    x = np.asarray(x, np.float32)
    gate_w = np.asarray(gate_w, np.float32)
    w1 = np.asarray(w1, np.float32)
    w2 = np.asarray(w2, np.float32)
    w3 = np.asarray(w3, np.float32)
    sw1 = np.asarray(sw1, np.float32)
    sw2 = np.asarray(sw2, np.float32)
    sw3 = np.asarray(sw3, np.float32)
    expert_bias = np.asarray(expert_bias, np.float32)
    BS, SLEN, D = x.shape
    T = BS * SLEN
    H = w1.shape[1]
    TS = T // NCORES
    KD = D // P
    KH = H // P
    MT_D = D // P
    XKC = 2
    KDC = KD // XKC
    up_chunks = _mchunks(H // P, WCH)
    dn_chunks = _mchunks(MT_D, WCH)
    msz_max = max(sz for _, sz in dn_chunks)

    xt, counts, tok_idx, scores_sorted = _route(x, gate_w, expert_bias)
    off = np.concatenate([[0], np.cumsum(counts)]).astype(np.int64)
    CAP = max(128, int(math.ceil(counts.max() / 128) * 128))
    NTOK = _pick_ntok(max(128, int(counts.max())), CAP)

    plan = _plan_segments(counts, NTOK)
    if plan is not None:
        SEGS, assign = plan
    else:
        SEGS = [NTOK]
        assign = [[(e, 0, int(counts[e]))] for e in range(NCORES)]
    NS = len(SEGS)
    FREE = [_free_div(S) for S in SEGS]
    NCH = [S // f for S, f in zip(SEGS, FREE)]
    FREE_S = _free_div(TS)
    NCH_S = TS // FREE_S

    key = (D, H, tuple(SEGS), TS)
    if key not in _PROGRAM_CACHE:
        _PROGRAM_CACHE[key] = _build_program(D, H, SEGS, TS)
    nc = _PROGRAM_CACHE[key]

    # ---- stage per-core inputs (swizzled; expert weight swizzles memoized
    # since a split expert appears on two cores) ----
    shared_chunks = {}
    for name, wmat, nk, chunks in (
        ("sw1", sw1.T, KD, up_chunks),
        ("sw3", sw3.T, KD, up_chunks),
        ("sw2", sw2.T, KH, dn_chunks),
    ):
        for c, arr in enumerate(_swz_w(np.ascontiguousarray(wmat), nk, chunks)):
            shared_chunks[f"{name}{c}"] = arr

    wmemo = {}

    def expert_chunks(kind, e):
        k = (kind, e)
        if k not in wmemo:
            mat, nk, chunks = {
                "w1": (w1[e].T, KD, up_chunks),
                "w3": (w3[e].T, KD, up_chunks),
                "w2": (w2[e].T, KH, dn_chunks),
            }[kind]
            wmemo[k] = _swz_w(np.ascontiguousarray(mat), nk, chunks)
        return wmemo[k]

    in_maps = []
    for core in range(NCORES):
        im = dict(shared_chunks)
        for s, (e, start, ln) in enumerate(assign[core]):
            seg = SEGS[s]
            idx = tok_idx[off[e] + start : off[e] + start + ln]
            xrT = np.zeros((D, seg), np.float32)
            xrT[:, :ln] = (
                xt[idx] * scores_sorted[off[e] + start : off[e] + start + ln, None]
            ).T
            for (n, kc), arr in _swz_x(xrT, KDC, XKC, FREE[s], NCH[s]).items():
                im[f"xr{s}_{n}_{kc}"] = arr
            for kind in ("w1", "w3", "w2"):
                for c, arr in enumerate(expert_chunks(kind, e)):
                    im[f"{kind}s{s}{c}"] = arr
        xsT = np.ascontiguousarray(xt[core * TS : (core + 1) * TS].T)
        im["xs"] = (
            xsT.reshape(KD, P, TS).transpose(1, 0, 2).reshape(P, KD * TS).astype(BF16)
        )
        in_maps.append(im)

    trace = os.environ.get("KERNEL_TRACE", "") not in ("", "0")
    if trace:
        _install_profhook()
    res = run_bass_kernel_spmd(nc, in_maps, list(range(NCORES)), trace=trace)
    LAST["exec_time_ns"] = res.exec_time_ns
    LAST["results"] = res

    # ---- combine: unswizzle outputs, shared slices + routed scatter-add ----
    def unswz(arr, nch, free):
        # arr [WCH, nch, P, msz_max*free] -> [D, nch*free] f32
        full = np.empty((MT_D * P, nch * free), np.float32)
        for c, (m0, sz) in enumerate(dn_chunks):
            blk = np.asarray(arr[c, :, :, : sz * free], dtype=np.float32).reshape(
                nch, P, sz, free
            )
            full[m0 * P : (m0 + sz) * P] = blk.transpose(2, 1, 0, 3).reshape(
                sz * P, nch * free
            )
        return full

    out = np.empty((T, D), np.float32)
    for core in range(NCORES):
        osw = np.asarray(res.results[core]["outs"])
        out[core * TS : (core + 1) * TS] = unswz(osw, NCH_S, FREE_S).T
    for core in range(NCORES):
        for s, (e, start, ln) in enumerate(assign[core]):
            if ln <= 0:
                continue
            orw = np.asarray(res.results[core][f"outr{s}"])
            full = unswz(orw, NCH[s], FREE[s])  # [D, SEGS[s]]
            idx = tok_idx[off[e] + start : off[e] + start + ln]
            out[idx] += full[:, :ln].T
    return out.reshape(BS, SLEN, D)
